# revision 10
# baseline (speedup 1.0000x reference)
"""AAM-Softmax (ArcFace) loss + top-1 accuracy on 8 TRN2 NeuronCores.

Math (reference): cosine = l2n(x) @ l2n(w).T ; the angular margin (phi) only
affects the label column; loss = mean CE of SCALE*logits;
prec1 = 100*mean(argmax==label).

Device restructuring:
- Margin math only needs the label column -> computed from w[label] rows
  (host-side row selection; all arithmetic on device).
- cosine in [-1,1] so exp(15*cos) fits fp32: no row-max pass. One sweep
  accumulates per-shard sum(exp) (ScalarE accum_out) and count(cos > phi_i)
  (VectorE fused compare+reduce); argmax==label <=> count==1.
- Classes sharded 8x (6250/core). Partials AllReduced in three staged 16KB
  collectives so all but the last (tiny) one hide behind compute.
- x-norm folded into the exp as a per-partition ScalarE scale (matmul uses
  raw bf16 x); count threshold tau = exp(15*phi) compared against the bf16
  exp tile (threshold sits in a probability desert, so bf16 rounding is safe).
- w rows normalized on device in natural layout; normalized tiles bounce
  through DRAM so the K-major transpose is 4 large XBAR DMA-transposes per
  2048-class block (per-call overhead ~1.2us dominates small transposes).
- rsqrt via exp(-0.5*ln) and an activation-table patch keep ScalarE on ONE
  table set (no ACT_TABLE_LOAD thrash).
- DMA work spread across queues: W loads + transposes on SyncE, x transposes
  on ScalarE's DGE, wnb stores + collectives on GpSimdE.
"""

import math
import sys

import numpy as np

if "/opt/trn_rl_repo" not in sys.path:
    sys.path.insert(0, "/opt/trn_rl_repo")

import ml_dtypes

N_CORES = 8
B, D, C = 2048, 512, 50000
CPS = C // N_CORES          # classes per shard: 6250
GW = 2048                   # mega-block width (classes)
NG = 3                      # full mega blocks: 3*2048 = 6144
TAILW = CPS - NG * GW       # 106
PAD_CPS = NG * GW + 128     # 6272 (tail padded to 128 rows)
MT = B // 128               # 16 batch tiles
KC = D // 128               # 4 k-chunks
NSUB = GW // 512            # 4 psum-bank-wide subtiles per mega block

MARGIN = 0.3
SCALE = 15.0
COS_M = math.cos(MARGIN)
SIN_M = math.sin(MARGIN)
TH = math.cos(math.pi - MARGIN)
MM = math.sin(math.pi - MARGIN) * MARGIN

_CACHE = {}


def _patch_act_tables():
    """Make natural_log_exp_and_others the only set offering Exp/Ln/Square,
    so bacc's table-load pass never ping-pongs between sets."""
    import concourse.bacc as bacc_mod
    import concourse.hw_specs as hw_specs
    from concourse import mybir

    if getattr(bacc_mod, "_aam_table_patch", False):
        return
    AF = mybir.ActivationFunctionType
    orig = hw_specs.get_activation_tables
    steal = {AF.Exp, AF.Ln, AF.Square, AF.Sign}
    target = "natural_log_exp_and_others"

    def patched(arch):
        t = orig(arch)
        return {
            name: (fns if name == target else fns - steal)
            for name, fns in t.items()
        }

    bacc_mod.get_activation_tables = patched
    bacc_mod._aam_table_patch = True


def _build():
    from concourse import bacc, mybir
    import concourse.tile as tile

    _patch_act_tables()

    f32 = mybir.dt.float32
    bf = mybir.dt.bfloat16
    AF = mybir.ActivationFunctionType
    OP = mybir.AluOpType
    AX = mybir.AxisListType.X
    RG = [list(range(N_CORES))]

    nc = bacc.Bacc("TRN2", target_bir_lowering=False, debug=False,
                   enable_asserts=False, num_devices=N_CORES)

    xb_d = nc.dram_tensor("xb", [B, D], bf, kind="ExternalInput").ap()
    ws_d = nc.dram_tensor("wshard", [PAD_CPS, D], bf, kind="ExternalInput").ap()
    wt_d = nc.dram_tensor("wtab", [B, D], bf, kind="ExternalInput").ap()
    out_d = nc.dram_tensor("out", [1, 2], f32, kind="ExternalOutput").ap()

    with tile.TileContext(nc) as tc:
        with tc.tile_pool(name="persist", bufs=1) as per, \
             tc.tile_pool(name="stage", bufs=3) as stage, \
             tc.tile_pool(name="wt", bufs=2) as wpool, \
             tc.tile_pool(name="scr", bufs=3) as scr, \
             tc.tile_pool(name="expool", bufs=6) as expool, \
             tc.tile_pool(name="xst", bufs=4) as xst, \
             tc.tile_pool(name="psum", bufs=2, space="PSUM") as psum, \
             tc.tile_pool(name="dram", bufs=2, space="DRAM") as dram:

            # persistent buffers
            xT = per.tile([128, KC, B], bf, tag="xT")
            # g0/g1 partials (col = m*2+g); g2 and tail accumulate straight
            # into their AllReduce input buffers.
            sums01 = per.tile([128, MT * 2], f32, tag="sums01")
            cnts01 = per.tile([128, MT * 2], f32, tag="cnts01")
            arin1 = per.tile([128, 32], f32, tag="arin1")
            arin2 = per.tile([128, 32], f32, tag="arin2")
            arin3 = per.tile([128, 32], f32, tag="arin3")
            ssqx = per.tile([128, MT], f32, tag="ssqx")
            ssqt = per.tile([128, MT], f32, tag="ssqt")
            dotr = per.tile([128, MT], f32, tag="dotr")
            invx = per.tile([128, MT], f32, tag="invx")
            s15x = per.tile([128, MT], f32, tag="s15x")
            tau = per.tile([128, MT], f32, tag="tau")
            phi15 = per.tile([128, MT], f32, tag="phi15")
            elab = per.tile([128, MT], f32, tag="elab")
            ones = per.tile([128, 1], f32, tag="ones")
            nc.vector.memset(ones[:], 1.0)

            # x transposed to K-major, on ScalarE's DGE queue (SyncE does W)
            for c in range(KC):
                nc.scalar.dma_start_transpose(
                    out=xT[:, c, :], in_=xb_d[:, c * 128:(c + 1) * 128])

            # ---- W-prep, one mega block at a time (g=0 before anything else
            # so the PE starts early; later blocks interleaved into the sweep)
            wT_tiles = {}

            def w_prep(g):
                tail = (g == NG)
                gw = 128 if tail else GW
                nsub = 1 if tail else NSUB
                wT = wpool.tile([128, KC, GW], bf, tag="wT")
                wT_tiles[g] = wT
                wnb_dram = dram.tile([GW, D], bf)
                for s in range(nsub):
                    wst = stage.tile([128, 4, D], bf, tag="wstage")
                    jn = 1 if tail else 4
                    for j in range(jn):
                        r0 = g * GW + s * 512 + j * 128
                        nc.sync.dma_start(out=wst[:, j, :],
                                          in_=ws_d[r0:r0 + 128, :])
                    sqw = scr.tile([128, 4, D], bf, tag="sqw")
                    nc.vector.tensor_tensor(out=sqw[:, :jn, :], in0=wst[:, :jn, :],
                                            in1=wst[:, :jn, :], op=OP.mult)
                    ssw = scr.tile([128, 4], f32, tag="ssw")
                    nc.vector.reduce_sum(out=ssw[:, :jn], in_=sqw[:, :jn, :], axis=AX)
                    ssc = scr.tile([128, 4], f32, tag="ssc")
                    nc.vector.tensor_scalar_max(ssc[:, :jn], ssw[:, :jn], 1e-12)
                    lnw = scr.tile([128, 4], f32, tag="lnw")
                    nc.scalar.activation(lnw[:, :jn], ssc[:, :jn], AF.Ln)
                    invw = scr.tile([128, 4], f32, tag="invw")
                    nc.scalar.activation(invw[:, :jn], lnw[:, :jn], AF.Exp, scale=-0.5)
                    wnb = scr.tile([128, 4, D], bf, tag="wnb")
                    for j in range(jn):
                        nc.scalar.activation(wnb[:, j, :], wst[:, j, :], AF.Copy,
                                             scale=invw[:, j:j + 1])
                        nc.gpsimd.dma_start(
                            out=wnb_dram[s * 512 + j * 128:s * 512 + (j + 1) * 128, :],
                            in_=wnb[:, j, :])
                for c in range(KC):
                    nc.sync.dma_start_transpose(
                        out=wT[:, c, :gw],
                        in_=wnb_dram[:gw, c * 128:(c + 1) * 128])

            w_prep(0)

            # ---------------- prologue stats (batched on DVE) -----------------
            # x squares first: they gate s15x -> the sweep's exp.
            xs_tiles = []
            for q in range(4):
                xs = xst.tile([128, 4, D], bf, tag="xstage")
                xs_tiles.append(xs)
                for j in range(4):
                    nc.sync.dma_start(
                        out=xs[:, j, :],
                        in_=xb_d[(q * 4 + j) * 128:(q * 4 + j + 1) * 128, :])
                sqx = scr.tile([128, 4, D], bf, tag="sqx")
                nc.vector.tensor_tensor(out=sqx[:], in0=xs[:], in1=xs[:], op=OP.mult)
                nc.vector.reduce_sum(out=ssqx[:, q * 4:(q + 1) * 4], in_=sqx[:],
                                     axis=AX)
            lnx = per.tile([128, MT], f32, tag="lnx")
            nc.scalar.activation(lnx[:], ssqx[:], AF.Ln)
            nc.scalar.activation(invx[:], lnx[:], AF.Exp, scale=-0.5)
            nc.vector.tensor_scalar_mul(s15x[:], invx[:], SCALE)

            # wtab squares + x.wtab dots
            for q in range(4):
                ts = stage.tile([128, 4, D], bf, tag="tstage")
                for j in range(4):
                    nc.sync.dma_start(
                        out=ts[:, j, :],
                        in_=wt_d[(q * 4 + j) * 128:(q * 4 + j + 1) * 128, :])
                sqt = scr.tile([128, 4, D], bf, tag="sqx")
                nc.vector.tensor_tensor(out=sqt[:], in0=ts[:], in1=ts[:], op=OP.mult)
                nc.vector.reduce_sum(out=ssqt[:, q * 4:(q + 1) * 4], in_=sqt[:],
                                     axis=AX)
                sqd = scr.tile([128, 4, D], bf, tag="sqx")
                nc.vector.tensor_tensor(out=sqd[:], in0=xs_tiles[q][:], in1=ts[:],
                                        op=OP.mult)
                nc.vector.reduce_sum(out=dotr[:, q * 4:(q + 1) * 4], in_=sqd[:],
                                     axis=AX)

            sst2 = per.tile([128, MT], f32, tag="sst2")
            nc.vector.tensor_scalar_max(sst2[:], ssqt[:], 1e-12)
            lnt = per.tile([128, MT], f32, tag="lnt")
            nc.scalar.activation(lnt[:], sst2[:], AF.Ln)
            invt = per.tile([128, MT], f32, tag="invt")
            nc.scalar.activation(invt[:], lnt[:], AF.Exp, scale=-0.5)

            # cos_lab = dotr * invx * invt
            tmp0 = per.tile([128, MT], f32, tag="tmp0")
            nc.vector.tensor_tensor(out=tmp0[:], in0=dotr[:], in1=invx[:], op=OP.mult)
            clab = per.tile([128, MT], f32, tag="clab")
            nc.vector.tensor_tensor(out=clab[:], in0=tmp0[:], in1=invt[:], op=OP.mult)

            # sine = sqrt(max(1-c^2,eps)) via exp(0.5*ln)
            c2 = per.tile([128, MT], f32, tag="c2")
            nc.vector.tensor_tensor(out=c2[:], in0=clab[:], in1=clab[:], op=OP.mult)
            s2 = per.tile([128, MT], f32, tag="s2")
            nc.vector.tensor_scalar(out=s2[:], in0=c2[:], scalar1=-1.0, scalar2=1.0,
                                    op0=OP.mult, op1=OP.add)
            s2c = per.tile([128, MT], f32, tag="s2c")
            nc.vector.tensor_scalar_max(s2c[:], s2[:], 1e-12)
            lns = per.tile([128, MT], f32, tag="lns")
            nc.scalar.activation(lns[:], s2c[:], AF.Ln)
            sine = per.tile([128, MT], f32, tag="sine")
            nc.scalar.activation(sine[:], lns[:], AF.Exp, scale=0.5)

            # phi = where(clab > TH, clab*COS_M - sine*SIN_M, clab - MM)
            pa = per.tile([128, MT], f32, tag="pa")
            nc.vector.tensor_scalar_mul(pa[:], clab[:], COS_M)
            pb = per.tile([128, MT], f32, tag="pb")
            nc.vector.tensor_scalar_mul(pb[:], sine[:], SIN_M)
            phi_m = per.tile([128, MT], f32, tag="phi_m")
            nc.vector.tensor_tensor(out=phi_m[:], in0=pa[:], in1=pb[:], op=OP.subtract)
            alt = per.tile([128, MT], f32, tag="alt")
            nc.vector.tensor_scalar_add(alt[:], clab[:], -MM)
            mask = per.tile([128, MT], f32, tag="mask")
            nc.vector.tensor_scalar(out=mask[:], in0=clab[:], scalar1=TH, scalar2=None,
                                    op0=OP.is_gt)
            dphi = per.tile([128, MT], f32, tag="dphi")
            nc.vector.tensor_tensor(out=dphi[:], in0=phi_m[:], in1=alt[:],
                                    op=OP.subtract)
            mdp = per.tile([128, MT], f32, tag="mdp")
            nc.vector.tensor_tensor(out=mdp[:], in0=mask[:], in1=dphi[:], op=OP.mult)
            phi_s = per.tile([128, MT], f32, tag="phi_s")
            nc.vector.tensor_tensor(out=phi_s[:], in0=alt[:], in1=mdp[:], op=OP.add)
            nc.vector.tensor_scalar_mul(phi15[:], phi_s[:], SCALE)
            nc.scalar.activation(tau[:], phi15[:], AF.Exp)
            nc.scalar.activation(elab[:], clab[:], AF.Exp, scale=SCALE)

            # ---------------- main sweep over mega blocks ---------------------
            for g in range(NG + 1):
                tail = (g == NG)
                gw = 128 if tail else GW           # padded width
                nw = TAILW if tail else GW         # valid width
                nsub = 1 if tail else NSUB
                wT = wT_tiles[g]
                if g == 0:
                    sacc, cacc, coff = sums01, cnts01, (lambda m: m * 2 + 0)
                elif g == 1:
                    sacc, cacc, coff = sums01, cnts01, (lambda m: m * 2 + 1)
                elif g == 2:
                    sacc, cacc, coff = arin2, arin2, (lambda m: m)
                else:
                    sacc, cacc, coff = arin3, arin3, (lambda m: m)
                for m in range(MT):
                    if m == 1 and g < NG:
                        w_prep(g + 1)
                    ms = slice(m * 128, (m + 1) * 128)
                    ps = psum.tile([128, GW], f32, tag="ps")
                    sw = gw if tail else 512
                    for s in range(nsub):
                        for c in range(KC):
                            nc.tensor.matmul(
                                ps[:, s * 512:s * 512 + sw],
                                lhsT=xT[:, c, ms],
                                rhs=wT[:, c, s * 512:s * 512 + sw],
                                start=(c == 0), stop=(c == KC - 1))
                    scol = coff(m)
                    ccol = scol if g < 2 else scol + 16
                    ex = expool.tile([128, GW], bf, tag="ex")
                    nc.scalar.activation(ex[:, :nw], ps[:, :nw], AF.Exp,
                                         scale=s15x[:, m:m + 1],
                                         accum_out=sacc[:, scol:scol + 1])
                    cn = scr.tile([128, GW], bf, tag="cn")
                    nc.vector.tensor_scalar(
                        out=cn[:, :nw], in0=ex[:, :nw], scalar1=tau[:, m:m + 1],
                        scalar2=None, op0=OP.is_gt, op1=OP.add,
                        accum_out=cacc[:, ccol:ccol + 1])

                if g == 1:
                    # AllReduce #1: g0+g1 partials, hidden behind mega 2
                    nc.vector.reduce_sum(
                        out=arin1[:, 0:16],
                        in_=sums01[:].rearrange("p (m g) -> p m g", g=2), axis=AX)
                    nc.vector.reduce_sum(
                        out=arin1[:, 16:32],
                        in_=cnts01[:].rearrange("p (m g) -> p m g", g=2), axis=AX)
                    cin1 = dram.tile([128, 32], f32)
                    cout1 = dram.tile([128, 32], f32, addr_space="Shared")
                    nc.gpsimd.dma_start(out=cin1[:], in_=arin1[:])
                    nc.gpsimd.collective_compute(
                        "AllReduce", OP.add, replica_groups=RG,
                        ins=[cin1[:]], outs=[cout1[:]])
                elif g == 2:
                    # AllReduce #2: g2 partials, hidden behind the tail block
                    cin2 = dram.tile([128, 32], f32)
                    cout2 = dram.tile([128, 32], f32, addr_space="Shared")
                    nc.gpsimd.dma_start(out=cin2[:], in_=arin2[:])
                    nc.gpsimd.collective_compute(
                        "AllReduce", OP.add, replica_groups=RG,
                        ins=[cin2[:]], outs=[cout2[:]])

            # AllReduce #3: tail partials (the only one on the critical path)
            cin3 = dram.tile([128, 32], f32)
            cout3 = dram.tile([128, 32], f32, addr_space="Shared")
            nc.gpsimd.dma_start(out=cin3[:], in_=arin3[:])
            nc.gpsimd.collective_compute(
                "AllReduce", OP.add, replica_groups=RG,
                ins=[cin3[:]], outs=[cout3[:]])

            tot1 = per.tile([128, 32], f32, tag="tot1")
            nc.sync.dma_start(out=tot1[:], in_=cout1[:])
            tot2 = per.tile([128, 32], f32, tag="tot2")
            nc.sync.dma_start(out=tot2[:], in_=cout2[:])
            tot3 = per.tile([128, 32], f32, tag="tot3")
            nc.sync.dma_start(out=tot3[:], in_=cout3[:])
            tot12 = per.tile([128, 32], f32, tag="tot12")
            nc.vector.tensor_tensor(out=tot12[:], in0=tot1[:], in1=tot2[:], op=OP.add)
            tot = per.tile([128, 32], f32, tag="tot")
            nc.vector.tensor_tensor(out=tot[:], in0=tot12[:], in1=tot3[:], op=OP.add)

            # ---------------- final scalars ------------------------------------
            sp1 = per.tile([128, MT], f32, tag="sp1")
            nc.vector.tensor_tensor(out=sp1[:], in0=tot[:, 0:16], in1=elab[:],
                                    op=OP.subtract)
            sp2 = per.tile([128, MT], f32, tag="sp2")
            nc.vector.tensor_tensor(out=sp2[:], in0=sp1[:], in1=tau[:], op=OP.add)
            lnS = per.tile([128, MT], f32, tag="lnS")
            nc.scalar.activation(lnS[:], sp2[:], AF.Ln)
            nll = per.tile([128, MT], f32, tag="nll")
            nc.vector.tensor_tensor(out=nll[:], in0=lnS[:], in1=phi15[:],
                                    op=OP.subtract)
            pack = per.tile([128, 2], f32, tag="pack")
            nc.vector.reduce_sum(out=pack[:, 0:1], in_=nll[:], axis=AX)
            corr = per.tile([128, MT], f32, tag="corr")
            nc.vector.tensor_scalar(out=corr[:], in0=tot[:, 16:32], scalar1=1.0,
                                    scalar2=None, op0=OP.is_equal)
            nc.vector.reduce_sum(out=pack[:, 1:2], in_=corr[:], axis=AX)
            fin = psum.tile([1, 2], f32, tag="ps")
            nc.tensor.matmul(fin[:], lhsT=ones[:], rhs=pack[:], start=True, stop=True)
            osb = per.tile([1, 2], f32, tag="osb")
            nc.scalar.mul(osb[:, 0:1], fin[:, 0:1], 1.0 / B)
            nc.scalar.mul(osb[:, 1:2], fin[:, 1:2], 100.0 / B)
            nc.sync.dma_start(out=out_d[:], in_=osb[:])

    nc.compile()
    return nc


def _get_nc():
    if "nc" not in _CACHE:
        _CACHE["nc"] = _build()
    return _CACHE["nc"]


def kernel(x: np.ndarray, weight: np.ndarray, label: np.ndarray, **_ignored):
    from concourse.bass_utils import run_bass_kernel_spmd

    bf = ml_dtypes.bfloat16
    x = np.ascontiguousarray(np.asarray(x, dtype=np.float32))
    weight = np.ascontiguousarray(np.asarray(weight, dtype=np.float32))
    lab = np.asarray(label).astype(np.int64)

    xb = x.astype(bf)
    wtab = weight[lab].astype(bf)
    in_maps = []
    for k in range(N_CORES):
        shard = np.zeros((PAD_CPS, D), dtype=bf)
        shard[:CPS] = weight[k * CPS:(k + 1) * CPS].astype(bf)
        in_maps.append({"xb": xb, "wshard": shard, "wtab": wtab})

    nc = _get_nc()
    res = run_bass_kernel_spmd(nc, in_maps, core_ids=list(range(N_CORES)))
    out = res.results[0]["out"]
    loss = np.float32(out[0, 0])
    prec1 = np.float32(out[0, 1])
    return (loss, prec1)


if __name__ == "__main__":
    pass


# revision 11
# speedup vs baseline: 1.0422x; 1.0422x over previous
"""AAM-Softmax (ArcFace) loss + top-1 accuracy on 8 TRN2 NeuronCores.

Math (reference): cosine = l2n(x) @ l2n(w).T ; the angular margin (phi) only
affects the label column; loss = mean CE of SCALE*logits;
prec1 = 100*mean(argmax==label).

Device restructuring:
- Margin math only needs the label column -> computed from w[label] rows
  (host-side row selection; all arithmetic on device).
- cosine in [-1,1] so exp(15*cos) fits fp32: no row-max pass. One sweep
  accumulates per-shard sum(exp) (ScalarE accum_out) and count(cos > phi_i)
  (VectorE fused compare+reduce); argmax==label <=> count==1.
- Classes sharded 8x (6250/core), swept in blocks [512,1536,2048,2048,106]:
  the small first block gets the TensorE streaming within ~20us.
- Partials AllReduced in three staged 16KB collectives so all but the last
  (tiny) one hide behind compute.
- x-norm folded into the exp as a per-partition ScalarE scale (matmul uses
  raw bf16 x); count threshold tau = exp(15*phi) compared against the bf16
  exp tile (threshold sits in a probability desert, so bf16 rounding is safe).
- w rows normalized on device in natural layout; normalized tiles bounce
  through DRAM so the K-major transpose is a few large XBAR DMA-transposes
  per block (per-call overhead ~1.2us dominates small transposes).
- rsqrt via exp(-0.5*ln) and an activation-table patch keep ScalarE on ONE
  table set (no ACT_TABLE_LOAD thrash).
- DMA spread across queues: W loads + all transposes on SyncE; x/wtab stage
  loads, wnb stores and collectives on GpSimdE's queue.
"""

import math
import sys

import numpy as np

if "/opt/trn_rl_repo" not in sys.path:
    sys.path.insert(0, "/opt/trn_rl_repo")

import ml_dtypes

N_CORES = 8
B, D, C = 2048, 512, 50000
CPS = C // N_CORES          # classes per shard: 6250
MT = B // 128               # 16 batch tiles
KC = D // 128               # 4 k-chunks

BLK_START = [0, 512, 2048, 4096, 6144]
BLK_W = [512, 1536, 2048, 2048, 128]     # padded widths (matmul)
BLK_VALID = [512, 1536, 2048, 2048, CPS - 6144]  # valid widths (exp/count)
NBLK5 = len(BLK_W)
PAD_CPS = 6272
GWMAX = 2048

MARGIN = 0.3
SCALE = 15.0
COS_M = math.cos(MARGIN)
SIN_M = math.sin(MARGIN)
TH = math.cos(math.pi - MARGIN)
MM = math.sin(math.pi - MARGIN) * MARGIN

_CACHE = {}


def _patch_act_tables():
    """Make natural_log_exp_and_others the only set offering Exp/Ln/Square,
    so bacc's table-load pass never ping-pongs between sets."""
    import concourse.bacc as bacc_mod
    import concourse.hw_specs as hw_specs
    from concourse import mybir

    if getattr(bacc_mod, "_aam_table_patch", False):
        return
    AF = mybir.ActivationFunctionType
    orig = hw_specs.get_activation_tables
    steal = {AF.Exp, AF.Ln, AF.Square, AF.Sign}
    target = "natural_log_exp_and_others"

    def patched(arch):
        t = orig(arch)
        return {
            name: (fns if name == target else fns - steal)
            for name, fns in t.items()
        }

    bacc_mod.get_activation_tables = patched
    bacc_mod._aam_table_patch = True


def _build():
    from concourse import bacc, mybir
    import concourse.tile as tile

    _patch_act_tables()

    f32 = mybir.dt.float32
    bf = mybir.dt.bfloat16
    AF = mybir.ActivationFunctionType
    OP = mybir.AluOpType
    AX = mybir.AxisListType.X
    RG = [list(range(N_CORES))]

    nc = bacc.Bacc("TRN2", target_bir_lowering=False, debug=False,
                   enable_asserts=False, num_devices=N_CORES)

    xb_d = nc.dram_tensor("xb", [B, D], bf, kind="ExternalInput").ap()
    ws_d = nc.dram_tensor("wshard", [PAD_CPS, D], bf, kind="ExternalInput").ap()
    wt_d = nc.dram_tensor("wtab", [B, D], bf, kind="ExternalInput").ap()
    out_d = nc.dram_tensor("out", [1, 2], f32, kind="ExternalOutput").ap()

    with tile.TileContext(nc) as tc:
        with tc.tile_pool(name="persist", bufs=1) as per, \
             tc.tile_pool(name="stage", bufs=3) as stage, \
             tc.tile_pool(name="wt", bufs=2) as wpool, \
             tc.tile_pool(name="scr", bufs=3) as scr, \
             tc.tile_pool(name="expool", bufs=6) as expool, \
             tc.tile_pool(name="xst", bufs=4) as xst, \
             tc.tile_pool(name="psum", bufs=2, space="PSUM") as psum, \
             tc.tile_pool(name="dram", bufs=2, space="DRAM") as dram:

            # persistent buffers
            xT = per.tile([128, KC, B], bf, tag="xT")
            # blocks 0-2 partials (col = m*3+b); blocks 3,4 accumulate straight
            # into their AllReduce input buffers.
            sums012 = per.tile([128, MT * 3], f32, tag="sums012")
            cnts012 = per.tile([128, MT * 3], f32, tag="cnts012")
            arin1 = per.tile([128, 32], f32, tag="arin1")
            arin2 = per.tile([128, 32], f32, tag="arin2")
            arin3 = per.tile([128, 32], f32, tag="arin3")
            ssqx = per.tile([128, MT], f32, tag="ssqx")
            ssqt = per.tile([128, MT], f32, tag="ssqt")
            dotr = per.tile([128, MT], f32, tag="dotr")
            invx = per.tile([128, MT], f32, tag="invx")
            s15x = per.tile([128, MT], f32, tag="s15x")
            tau = per.tile([128, MT], f32, tag="tau")
            phi15 = per.tile([128, MT], f32, tag="phi15")
            elab = per.tile([128, MT], f32, tag="elab")
            ones = per.tile([128, 1], f32, tag="ones")
            nc.vector.memset(ones[:], 1.0)

            wT_tiles = {}

            def w_prep(b):
                bs, gw = BLK_START[b], BLK_W[b]
                nsub = (gw + 511) // 512
                wT = wpool.tile([128, KC, GWMAX], bf, tag="wT")
                wT_tiles[b] = wT
                wnb_dram = dram.tile([GWMAX, D], bf)
                for s in range(nsub):
                    jn = min(4, (gw - s * 512 + 127) // 128)
                    wst = stage.tile([128, 4, D], bf, tag="wstage")
                    for j in range(jn):
                        r0 = bs + s * 512 + j * 128
                        nc.sync.dma_start(out=wst[:, j, :],
                                          in_=ws_d[r0:r0 + 128, :])
                    if b == 0:
                        # first block: interleave x transposes right after the
                        # first W loads on the same queue
                        for c in range(KC):
                            nc.sync.dma_start_transpose(
                                out=xT[:, c, :],
                                in_=xb_d[:, c * 128:(c + 1) * 128])
                    sqw = scr.tile([128, 4, D], bf, tag="sqw")
                    nc.vector.tensor_tensor(out=sqw[:, :jn, :], in0=wst[:, :jn, :],
                                            in1=wst[:, :jn, :], op=OP.mult)
                    ssw = scr.tile([128, 4], f32, tag="ssw")
                    nc.vector.reduce_sum(out=ssw[:, :jn], in_=sqw[:, :jn, :], axis=AX)
                    ssc = scr.tile([128, 4], f32, tag="ssc")
                    nc.vector.tensor_scalar_max(ssc[:, :jn], ssw[:, :jn], 1e-12)
                    lnw = scr.tile([128, 4], f32, tag="lnw")
                    nc.scalar.activation(lnw[:, :jn], ssc[:, :jn], AF.Ln)
                    invw = scr.tile([128, 4], f32, tag="invw")
                    nc.scalar.activation(invw[:, :jn], lnw[:, :jn], AF.Exp, scale=-0.5)
                    wnb = scr.tile([128, 4, D], bf, tag="wnb")
                    for j in range(jn):
                        nc.scalar.activation(wnb[:, j, :], wst[:, j, :], AF.Copy,
                                             scale=invw[:, j:j + 1])
                        nc.gpsimd.dma_start(
                            out=wnb_dram[s * 512 + j * 128:s * 512 + (j + 1) * 128, :],
                            in_=wnb[:, j, :])
                for c in range(KC):
                    nc.sync.dma_start_transpose(
                        out=wT[:, c, :gw],
                        in_=wnb_dram[:gw, c * 128:(c + 1) * 128])

            w_prep(0)

            # ---------------- prologue stats (batched on DVE) -----------------
            # x squares first: they gate s15x -> the sweep's exp.
            xs_tiles = []
            for q in range(4):
                xs = xst.tile([128, 4, D], bf, tag="xstage")
                xs_tiles.append(xs)
                for j in range(4):
                    nc.gpsimd.dma_start(
                        out=xs[:, j, :],
                        in_=xb_d[(q * 4 + j) * 128:(q * 4 + j + 1) * 128, :])
                sqx = scr.tile([128, 4, D], bf, tag="sqx")
                nc.vector.tensor_tensor(out=sqx[:], in0=xs[:], in1=xs[:], op=OP.mult)
                nc.vector.reduce_sum(out=ssqx[:, q * 4:(q + 1) * 4], in_=sqx[:],
                                     axis=AX)
            lnx = per.tile([128, MT], f32, tag="lnx")
            nc.scalar.activation(lnx[:], ssqx[:], AF.Ln)
            nc.scalar.activation(invx[:], lnx[:], AF.Exp, scale=-0.5)
            nc.vector.tensor_scalar_mul(s15x[:], invx[:], SCALE)

            # wtab squares + x.wtab dots
            for q in range(4):
                ts = stage.tile([128, 4, D], bf, tag="tstage")
                for j in range(4):
                    nc.gpsimd.dma_start(
                        out=ts[:, j, :],
                        in_=wt_d[(q * 4 + j) * 128:(q * 4 + j + 1) * 128, :])
                sqt = scr.tile([128, 4, D], bf, tag="sqx")
                nc.vector.tensor_tensor(out=sqt[:], in0=ts[:], in1=ts[:], op=OP.mult)
                nc.vector.reduce_sum(out=ssqt[:, q * 4:(q + 1) * 4], in_=sqt[:],
                                     axis=AX)
                sqd = scr.tile([128, 4, D], bf, tag="sqx")
                nc.vector.tensor_tensor(out=sqd[:], in0=xs_tiles[q][:], in1=ts[:],
                                        op=OP.mult)
                nc.vector.reduce_sum(out=dotr[:, q * 4:(q + 1) * 4], in_=sqd[:],
                                     axis=AX)

            sst2 = per.tile([128, MT], f32, tag="sst2")
            nc.vector.tensor_scalar_max(sst2[:], ssqt[:], 1e-12)
            lnt = per.tile([128, MT], f32, tag="lnt")
            nc.scalar.activation(lnt[:], sst2[:], AF.Ln)
            invt = per.tile([128, MT], f32, tag="invt")
            nc.scalar.activation(invt[:], lnt[:], AF.Exp, scale=-0.5)

            # cos_lab = dotr * invx * invt
            tmp0 = per.tile([128, MT], f32, tag="tmp0")
            nc.vector.tensor_tensor(out=tmp0[:], in0=dotr[:], in1=invx[:], op=OP.mult)
            clab = per.tile([128, MT], f32, tag="clab")
            nc.vector.tensor_tensor(out=clab[:], in0=tmp0[:], in1=invt[:], op=OP.mult)

            # sine = sqrt(max(1-c^2,eps)) via exp(0.5*ln)
            c2 = per.tile([128, MT], f32, tag="c2")
            nc.vector.tensor_tensor(out=c2[:], in0=clab[:], in1=clab[:], op=OP.mult)
            s2 = per.tile([128, MT], f32, tag="s2")
            nc.vector.tensor_scalar(out=s2[:], in0=c2[:], scalar1=-1.0, scalar2=1.0,
                                    op0=OP.mult, op1=OP.add)
            s2c = per.tile([128, MT], f32, tag="s2c")
            nc.vector.tensor_scalar_max(s2c[:], s2[:], 1e-12)
            lns = per.tile([128, MT], f32, tag="lns")
            nc.scalar.activation(lns[:], s2c[:], AF.Ln)
            sine = per.tile([128, MT], f32, tag="sine")
            nc.scalar.activation(sine[:], lns[:], AF.Exp, scale=0.5)

            # phi = where(clab > TH, clab*COS_M - sine*SIN_M, clab - MM)
            pa = per.tile([128, MT], f32, tag="pa")
            nc.vector.tensor_scalar_mul(pa[:], clab[:], COS_M)
            pb = per.tile([128, MT], f32, tag="pb")
            nc.vector.tensor_scalar_mul(pb[:], sine[:], SIN_M)
            phi_m = per.tile([128, MT], f32, tag="phi_m")
            nc.vector.tensor_tensor(out=phi_m[:], in0=pa[:], in1=pb[:], op=OP.subtract)
            alt = per.tile([128, MT], f32, tag="alt")
            nc.vector.tensor_scalar_add(alt[:], clab[:], -MM)
            mask = per.tile([128, MT], f32, tag="mask")
            nc.vector.tensor_scalar(out=mask[:], in0=clab[:], scalar1=TH, scalar2=None,
                                    op0=OP.is_gt)
            dphi = per.tile([128, MT], f32, tag="dphi")
            nc.vector.tensor_tensor(out=dphi[:], in0=phi_m[:], in1=alt[:],
                                    op=OP.subtract)
            mdp = per.tile([128, MT], f32, tag="mdp")
            nc.vector.tensor_tensor(out=mdp[:], in0=mask[:], in1=dphi[:], op=OP.mult)
            phi_s = per.tile([128, MT], f32, tag="phi_s")
            nc.vector.tensor_tensor(out=phi_s[:], in0=alt[:], in1=mdp[:], op=OP.add)
            nc.vector.tensor_scalar_mul(phi15[:], phi_s[:], SCALE)
            nc.scalar.activation(tau[:], phi15[:], AF.Exp)
            nc.scalar.activation(elab[:], clab[:], AF.Exp, scale=SCALE)

            # ---------------- main sweep over class blocks --------------------
            for b in range(NBLK5):
                gw, nw = BLK_W[b], BLK_VALID[b]
                nsub = (gw + 511) // 512
                wT = wT_tiles[b]
                for m in range(MT):
                    if m == 1 and b + 1 < NBLK5:
                        w_prep(b + 1)
                    ms = slice(m * 128, (m + 1) * 128)
                    ps = psum.tile([128, GWMAX], f32, tag="ps")
                    for s in range(nsub):
                        sw = min(512, gw - s * 512)
                        for c in range(KC):
                            nc.tensor.matmul(
                                ps[:, s * 512:s * 512 + sw],
                                lhsT=xT[:, c, ms],
                                rhs=wT[:, c, s * 512:s * 512 + sw],
                                start=(c == 0), stop=(c == KC - 1))
                    if b < 3:
                        sac, cac = sums012[:, m * 3 + b:m * 3 + b + 1], \
                                   cnts012[:, m * 3 + b:m * 3 + b + 1]
                    elif b == 3:
                        sac, cac = arin2[:, m:m + 1], arin2[:, 16 + m:17 + m]
                    else:
                        sac, cac = arin3[:, m:m + 1], arin3[:, 16 + m:17 + m]
                    ex = expool.tile([128, GWMAX], bf, tag="ex")
                    nc.scalar.activation(ex[:, :nw], ps[:, :nw], AF.Exp,
                                         scale=s15x[:, m:m + 1], accum_out=sac)
                    cn = scr.tile([128, GWMAX], bf, tag="cn")
                    nc.vector.tensor_scalar(
                        out=cn[:, :nw], in0=ex[:, :nw], scalar1=tau[:, m:m + 1],
                        scalar2=None, op0=OP.is_gt, op1=OP.add, accum_out=cac)

                if b == 2:
                    # AllReduce #1: blocks 0-2 partials, hidden behind block 3
                    nc.vector.reduce_sum(
                        out=arin1[:, 0:16],
                        in_=sums012[:].rearrange("p (m b) -> p m b", b=3), axis=AX)
                    nc.vector.reduce_sum(
                        out=arin1[:, 16:32],
                        in_=cnts012[:].rearrange("p (m b) -> p m b", b=3), axis=AX)
                    cin1 = dram.tile([128, 32], f32)
                    cout1 = dram.tile([128, 32], f32, addr_space="Shared")
                    nc.gpsimd.dma_start(out=cin1[:], in_=arin1[:])
                    nc.gpsimd.collective_compute(
                        "AllReduce", OP.add, replica_groups=RG,
                        ins=[cin1[:]], outs=[cout1[:]])
                elif b == 3:
                    # AllReduce #2: block 3 partials, hidden behind the tail
                    cin2 = dram.tile([128, 32], f32)
                    cout2 = dram.tile([128, 32], f32, addr_space="Shared")
                    nc.gpsimd.dma_start(out=cin2[:], in_=arin2[:])
                    nc.gpsimd.collective_compute(
                        "AllReduce", OP.add, replica_groups=RG,
                        ins=[cin2[:]], outs=[cout2[:]])

            # AllReduce #3: tail partials (the only one on the critical path)
            cin3 = dram.tile([128, 32], f32)
            cout3 = dram.tile([128, 32], f32, addr_space="Shared")
            nc.gpsimd.dma_start(out=cin3[:], in_=arin3[:])
            nc.gpsimd.collective_compute(
                "AllReduce", OP.add, replica_groups=RG,
                ins=[cin3[:]], outs=[cout3[:]])

            tot1 = per.tile([128, 32], f32, tag="tot1")
            nc.sync.dma_start(out=tot1[:], in_=cout1[:])
            tot2 = per.tile([128, 32], f32, tag="tot2")
            nc.sync.dma_start(out=tot2[:], in_=cout2[:])
            tot3 = per.tile([128, 32], f32, tag="tot3")
            nc.sync.dma_start(out=tot3[:], in_=cout3[:])
            tot12 = per.tile([128, 32], f32, tag="tot12")
            nc.vector.tensor_tensor(out=tot12[:], in0=tot1[:], in1=tot2[:], op=OP.add)
            tot = per.tile([128, 32], f32, tag="tot")
            nc.vector.tensor_tensor(out=tot[:], in0=tot12[:], in1=tot3[:], op=OP.add)

            # ---------------- final scalars ------------------------------------
            sp1 = per.tile([128, MT], f32, tag="sp1")
            nc.vector.tensor_tensor(out=sp1[:], in0=tot[:, 0:16], in1=elab[:],
                                    op=OP.subtract)
            sp2 = per.tile([128, MT], f32, tag="sp2")
            nc.vector.tensor_tensor(out=sp2[:], in0=sp1[:], in1=tau[:], op=OP.add)
            lnS = per.tile([128, MT], f32, tag="lnS")
            nc.scalar.activation(lnS[:], sp2[:], AF.Ln)
            nll = per.tile([128, MT], f32, tag="nll")
            nc.vector.tensor_tensor(out=nll[:], in0=lnS[:], in1=phi15[:],
                                    op=OP.subtract)
            pack = per.tile([128, 2], f32, tag="pack")
            nc.vector.reduce_sum(out=pack[:, 0:1], in_=nll[:], axis=AX)
            corr = per.tile([128, MT], f32, tag="corr")
            nc.vector.tensor_scalar(out=corr[:], in0=tot[:, 16:32], scalar1=1.0,
                                    scalar2=None, op0=OP.is_equal)
            nc.vector.reduce_sum(out=pack[:, 1:2], in_=corr[:], axis=AX)
            fin = psum.tile([1, 2], f32, tag="ps")
            nc.tensor.matmul(fin[:], lhsT=ones[:], rhs=pack[:], start=True, stop=True)
            osb = per.tile([1, 2], f32, tag="osb")
            nc.scalar.mul(osb[:, 0:1], fin[:, 0:1], 1.0 / B)
            nc.scalar.mul(osb[:, 1:2], fin[:, 1:2], 100.0 / B)
            nc.sync.dma_start(out=out_d[:], in_=osb[:])

    nc.compile()
    return nc


def _get_nc():
    if "nc" not in _CACHE:
        _CACHE["nc"] = _build()
    return _CACHE["nc"]


def kernel(x: np.ndarray, weight: np.ndarray, label: np.ndarray, **_ignored):
    from concourse.bass_utils import run_bass_kernel_spmd

    bf = ml_dtypes.bfloat16
    x = np.ascontiguousarray(np.asarray(x, dtype=np.float32))
    weight = np.ascontiguousarray(np.asarray(weight, dtype=np.float32))
    lab = np.asarray(label).astype(np.int64)

    xb = x.astype(bf)
    wtab = weight[lab].astype(bf)
    in_maps = []
    for k in range(N_CORES):
        shard = np.zeros((PAD_CPS, D), dtype=bf)
        shard[:CPS] = weight[k * CPS:(k + 1) * CPS].astype(bf)
        in_maps.append({"xb": xb, "wshard": shard, "wtab": wtab})

    nc = _get_nc()
    res = run_bass_kernel_spmd(nc, in_maps, core_ids=list(range(N_CORES)))
    out = res.results[0]["out"]
    loss = np.float32(out[0, 0])
    prec1 = np.float32(out[0, 1])
    return (loss, prec1)


if __name__ == "__main__":
    pass
